# revision 13
# baseline (speedup 1.0000x reference)
"""DiagSSMBlock Trainium2 kernel.

h_t = sum_{k=0..t} a^k * (B^T x_{t-k})  ==  h_t = a * h_{t-1} + s_t, s = B^T x^T.

Strategy: shard T across the 8 cores (1024 steps each + 8-step halo; |a| <=
sqrt(2/1024) ~ 0.044 so a^8 ~ 1.5e-11 — invisible at fp32, making slabs
independent).  bf16 operands keep the PE at 1 cycle/row (same rate as
fp32r) while halving HBM traffic.

The host repacks each core's entire input (a-vector, x^T slab, B) into ONE
[128, 16456] bf16 DRAM tensor laid out in exact consumption order:
[av | x0_kq0 b_kq0 | x0_kq1 b_kq1 | ... | x12_kq0..7].  Measured DMA costs
on this part: ~0.6 us of descriptor generation per 128-descriptor segment
plus ~2.1 us of completion serialization per dma_start on its ring — so
every load is a single-segment contiguous [128, range] slice, six loads
total, three per HWDGE ring, sized so arrival tracks the kq-major
consumption order.

Other layout decisions (from ntff traces):
  - ~7 us framework preamble; N_WARM dummy matmuls bridge preamble-end to
    first-data so the PE HAM clock-gate is at 2.4 GHz for the real stream
    (HAM holds through sub-3.4us gaps);
  - chunk ni=0 runs kq-MAJOR across all 8 PSUM banks (one per 128-channel
    group): passes start when the first (av,x0,b0,b1) slice lands and
    consume b rows roughly as they arrive; chunks 1-2 run g-major so each
    group's DVE scan frees its PSUM bank just ahead of reuse;
  - the SSM recurrence is DVE tensor_tensor_scan (1x only: no faster uop
    exists, and SBUF vs PSUM operands measure identically), per-partition
    `a` via stride-0 broadcast AP, bf16 output;
  - stores alternate rings in scan-completion order.
"""

import sys

if "/opt/trn_rl_repo" not in sys.path:
    sys.path.insert(0, "/opt/trn_rl_repo")

import ml_dtypes
import numpy as np

T, H = 8192, 1024
NC = 8
P = 128
T_LOC = T // NC            # 1024 output timesteps per core
HALO = 8                   # scan warmup; a^8 ~ 1.5e-11
W = T_LOC + HALO           # 1032
AV = 8                     # a-vector columns at the head of xcat
CH = 344                   # psum chunk width (3 chunks of 344 = 1032)
NCHUNK = W // CH           # 3
KQ = H // P                # 8 contraction chunks
G = H // P                 # 8 output-channel groups
N_WARM = 34                # dummy matmuls to lift the HAM clock gate

BLK = CH + H               # one kq block in xcat: x0 chunk + b row
XB = AV + KQ * BLK         # start of the x12 region
NCOL = XB + KQ * (W - CH)  # 16456

_state = {}


def _build_nc():
    import concourse.tile as tile
    from concourse import bacc, mybir

    bf16 = mybir.dt.bfloat16
    f32 = mybir.dt.float32

    nc = bacc.Bacc("TRN2", target_bir_lowering=False, debug=False, num_devices=NC)
    xc_e = nc.dram_tensor("xcat", [P, NCOL], bf16, kind="ExternalInput").ap()
    out_e = nc.dram_tensor("out", [H, T_LOC], bf16, kind="ExternalOutput").ap()
    flush_e = nc.dram_tensor("warm_flush", [P, 1], f32).ap()

    # six contiguous load pieces, alternating rings in consumption order
    piece_bounds = [
        0,           # sync   : av + kq0 + kq1
        2744,        # scalar : kq2 + kq3
        5480,        # sync   : kq4 + kq5
        8216,        # scalar : kq6 + kq7
        XB,          # sync   : x12 kq0-3   (XB == 10952)
        13704,       # scalar : x12 kq4-7
        NCOL,
    ]

    with tile.TileContext(nc) as tc:
        with (
            tc.tile_pool(name="consts", bufs=1) as consts,
            tc.tile_pool(name="xcpool", bufs=1) as xcpool,
            tc.tile_pool(name="hpool", bufs=1) as hpool,
            tc.tile_pool(name="pspool", bufs=8, space="PSUM") as pspool,
        ):
            # PE warm-up: bridge preamble-end -> first-data with dummy MMs.
            warm_sb = consts.tile([P, P], bf16, tag="warm")
            nc.gpsimd.memset(warm_sb[:], 0.0)
            wps = pspool.tile([P, P], f32, tag="ps", name="wps")
            for i in range(N_WARM):
                nc.tensor.matmul(
                    wps[:],
                    warm_sb[:],
                    warm_sb[:],
                    start=(i == 0),
                    stop=(i == N_WARM - 1),
                )
            flush_sb = consts.tile([P, 1], f32, tag="flush")
            nc.vector.tensor_copy(flush_sb[:], wps[:, 0:1])

            pieces = []
            for pi in range(6):
                c0, c1 = piece_bounds[pi], piece_bounds[pi + 1]
                eng = nc.sync if pi % 2 == 0 else nc.scalar
                t = xcpool.tile([P, c1 - c0], bf16, tag=f"pc{pi}", name=f"pc{pi}")
                eng.dma_start(t[:], xc_e[:, c0:c1])
                pieces.append((t, c0, c1))

            def col_slice(c0, n):
                for t, p0, p1 in pieces:
                    if p0 <= c0 and c0 + n <= p1:
                        return t[:, c0 - p0 : c0 - p0 + n]
                raise AssertionError(f"no piece covers [{c0}, {c0 + n})")

            a_op = [
                pieces[0][0][:, g : g + 1].broadcast_to([P, CH]) for g in range(G)
            ]

            def b_slice(kq, g):
                return col_slice(AV + kq * BLK + CH + g * P, P)

            def x_chunk(kq, ni):
                if ni == 0:
                    return col_slice(AV + kq * BLK, CH)
                return col_slice(XB + kq * (W - CH) + (ni - 1) * CH, CH)

            h_t = [
                hpool.tile([P, W], bf16, tag=f"h{g}", name=f"h{g}")
                for g in range(G)
            ]

            def scan_and_store(g, ni, ps_g):
                n0 = ni * CH
                init = 0.0 if ni == 0 else h_t[g][:, n0 - 1 : n0]
                nc.vector.tensor_tensor_scan(
                    h_t[g][:, n0 : n0 + CH],
                    a_op[g],
                    ps_g[:],
                    init,
                    op0=mybir.AluOpType.mult,
                    op1=mybir.AluOpType.add,
                )
                lo = HALO if ni == 0 else 0
                eng = nc.scalar if g % 2 == 0 else nc.sync
                eng.dma_start(
                    out_e[g * P : (g + 1) * P, n0 + lo - HALO : n0 + CH - HALO],
                    h_t[g][:, n0 + lo : n0 + CH],
                )

            # chunk 0: kq-major passes for kq 0-4 across all 8 PSUM banks
            # (DMA-arrival matched), then per-group (kq5,kq6,kq7) finishing
            # bursts so accumulation stops stagger ~0.44 us apart — the
            # first scan starts ~2 us earlier than with a full kq7 pass,
            # and the post-chunk-0 phase is DVE-bound.
            ps0 = [
                pspool.tile([P, CH], f32, tag="ps", name=f"ps0_{g}")
                for g in range(G)
            ]
            for kq in range(5):
                for g in range(G):
                    nc.tensor.matmul(
                        ps0[g][:],
                        b_slice(kq, g),
                        x_chunk(kq, 0),
                        start=(kq == 0),
                        stop=False,
                    )
            for g in range(G):
                for kq in (5, 6, 7):
                    nc.tensor.matmul(
                        ps0[g][:],
                        b_slice(kq, g),
                        x_chunk(kq, 0),
                        start=False,
                        stop=(kq == KQ - 1),
                    )
                scan_and_store(g, 0, ps0[g])

            # chunks 1-2: g-major, scans chase and free banks just in time
            for ni in (1, 2):
                for g in range(G):
                    ps_g = pspool.tile([P, CH], f32, tag="ps", name=f"ps{ni}_{g}")
                    for kq in range(KQ):
                        nc.tensor.matmul(
                            ps_g[:],
                            b_slice(kq, g),
                            x_chunk(kq, ni),
                            start=(kq == 0),
                            stop=(kq == KQ - 1),
                        )
                    scan_and_store(g, ni, ps_g)

            # warm-MM flush store, late, on scalar's ring (anti-DCE)
            nc.scalar.dma_start(flush_e[:], flush_sb[:])

    nc.compile()
    return nc


def _get_nc():
    if "nc" not in _state:
        _state["nc"] = _build_nc()
    return _state["nc"]


def _shard_inputs(x_seq, a_diag, b_mat):
    x = np.asarray(x_seq, dtype=np.float32)
    a = np.asarray(a_diag, dtype=np.float32)
    b = np.asarray(b_mat, dtype=np.float32)
    bq = b.astype(ml_dtypes.bfloat16)  # [H, H]
    x_pad = np.concatenate([np.zeros((HALO, H), np.float32), x], axis=0)
    xT = np.ascontiguousarray(x_pad.T).astype(ml_dtypes.bfloat16)  # [H, T+HALO]
    avT = np.ascontiguousarray(a.reshape(G, P).T).astype(ml_dtypes.bfloat16)

    in_maps = []
    for i in range(NC):
        slab = xT[:, i * T_LOC : i * T_LOC + W]  # [H, W]
        xcat = np.empty((P, NCOL), dtype=ml_dtypes.bfloat16)
        xcat[:, 0:AV] = avT
        for kq in range(KQ):
            r0, r1 = kq * P, (kq + 1) * P
            c = AV + kq * BLK
            xcat[:, c : c + CH] = slab[r0:r1, 0:CH]
            xcat[:, c + CH : c + BLK] = bq[r0:r1, :]
            xc = XB + kq * (W - CH)
            xcat[:, xc : xc + (W - CH)] = slab[r0:r1, CH:W]
        in_maps.append({"xcat": xcat})
    return in_maps


def kernel(x_seq, a_diag, b_mat):
    from concourse.bass_utils import run_bass_kernel_spmd

    nc = _get_nc()
    in_maps = _shard_inputs(x_seq, a_diag, b_mat)
    res = run_bass_kernel_spmd(nc, in_maps, list(range(NC)))
    _state["last_result"] = res
    out = np.concatenate(
        [
            np.asarray(res.results[i]["out"]).astype(np.float32).T
            for i in range(NC)
        ],
        axis=0,
    )
    return out


# revision 14
# speedup vs baseline: 1.0570x; 1.0570x over previous
"""DiagSSMBlock Trainium2 kernel.

h_t = sum_{k=0..t} a^k * (B^T x_{t-k})  ==  h_t = a * h_{t-1} + s_t, s = B^T x^T.

Strategy: shard T across the 8 cores (1024 steps each + 8-step halo; |a| <=
sqrt(2/1024) ~ 0.044 so a^8 ~ 1.5e-11 — invisible at fp32, making slabs
independent).  bf16 operands keep the PE at 1 cycle/row (same rate as
fp32r) while halving HBM traffic.

The host repacks each core's entire input (a-vector, x^T slab, B) into ONE
[128, 16456] bf16 DRAM tensor laid out in exact consumption order:
[av | x0_kq0 b_kq0 | x0_kq1 b_kq1 | ... | x12_kq0..7].  Measured DMA costs
on this part: ~0.6 us of descriptor generation per 128-descriptor segment
plus ~2.1 us of completion serialization per dma_start on its ring — so
every load is a single-segment contiguous [128, range] slice, six loads
total, three per HWDGE ring, sized so arrival tracks the kq-major
consumption order.

Other layout decisions (from ntff traces):
  - ~7 us framework preamble; N_WARM dummy matmuls bridge preamble-end to
    first-data so the PE HAM clock-gate is at 2.4 GHz for the real stream
    (HAM holds through sub-3.4us gaps);
  - chunk ni=0 runs kq-MAJOR across all 8 PSUM banks (one per 128-channel
    group): passes start when the first (av,x0,b0,b1) slice lands and
    consume b rows roughly as they arrive; chunks 1-2 run g-major so each
    group's DVE scan frees its PSUM bank just ahead of reuse;
  - the SSM recurrence is DVE tensor_tensor_scan (1x only: no faster uop
    exists, and SBUF vs PSUM operands measure identically), per-partition
    `a` via stride-0 broadcast AP, bf16 output;
  - stores alternate rings in scan-completion order.
"""

import sys

if "/opt/trn_rl_repo" not in sys.path:
    sys.path.insert(0, "/opt/trn_rl_repo")

import ml_dtypes
import numpy as np

T, H = 8192, 1024
NC = 8
P = 128
T_LOC = T // NC            # 1024 output timesteps per core
HALO = 8                   # scan warmup; a^8 ~ 1.5e-11
W = T_LOC + HALO           # 1032
AV = 8                     # a-vector columns at the head of xcat
CH = 344                   # psum chunk width (3 chunks of 344 = 1032)
NCHUNK = W // CH           # 3
KQ = H // P                # 8 contraction chunks
G = H // P                 # 8 output-channel groups
N_WARM = 34                # dummy matmuls to lift the HAM clock gate

BLK = CH + H               # one kq block in xcat: x0 chunk + b row
XB = AV + KQ * BLK         # start of the x12 region
NCOL = XB + KQ * (W - CH)  # 16456

_state = {}


def _build_nc():
    import concourse.tile as tile
    from concourse import bacc, mybir

    bf16 = mybir.dt.bfloat16
    f32 = mybir.dt.float32

    nc = bacc.Bacc("TRN2", target_bir_lowering=False, debug=False, num_devices=NC)
    xc_e = nc.dram_tensor("xcat", [P, NCOL], bf16, kind="ExternalInput").ap()
    out_e = nc.dram_tensor("out", [H, T_LOC], bf16, kind="ExternalOutput").ap()
    flush_e = nc.dram_tensor("warm_flush", [P, 1], f32).ap()

    # six contiguous load pieces, alternating rings in consumption order
    piece_bounds = [
        0,           # sync   : av + kq0 + kq1
        2744,        # scalar : kq2 + kq3
        5480,        # sync   : kq4 + kq5
        8216,        # scalar : kq6 + kq7
        XB,          # sync   : x12 kq0-3   (XB == 10952)
        13704,       # scalar : x12 kq4-7
        NCOL,
    ]

    with tile.TileContext(nc) as tc:
        with (
            tc.tile_pool(name="consts", bufs=1) as consts,
            tc.tile_pool(name="xcpool", bufs=1) as xcpool,
            tc.tile_pool(name="hpool", bufs=1) as hpool,
            tc.tile_pool(name="pspool", bufs=8, space="PSUM") as pspool,
        ):
            # PE warm-up: bridge preamble-end -> first-data with dummy MMs.
            warm_sb = consts.tile([P, P], bf16, tag="warm")
            nc.gpsimd.memset(warm_sb[:], 0.0)
            wps = pspool.tile([P, P], f32, tag="ps", name="wps")
            for i in range(N_WARM):
                nc.tensor.matmul(
                    wps[:],
                    warm_sb[:],
                    warm_sb[:],
                    start=(i == 0),
                    stop=(i == N_WARM - 1),
                )
            flush_sb = consts.tile([P, 1], f32, tag="flush")
            nc.vector.tensor_copy(flush_sb[:], wps[:, 0:1])

            pieces = []
            for pi in range(6):
                c0, c1 = piece_bounds[pi], piece_bounds[pi + 1]
                eng = nc.sync if pi % 2 == 0 else nc.scalar
                t = xcpool.tile([P, c1 - c0], bf16, tag=f"pc{pi}", name=f"pc{pi}")
                eng.dma_start(t[:], xc_e[:, c0:c1])
                pieces.append((t, c0, c1))

            def col_slice(c0, n):
                for t, p0, p1 in pieces:
                    if p0 <= c0 and c0 + n <= p1:
                        return t[:, c0 - p0 : c0 - p0 + n]
                raise AssertionError(f"no piece covers [{c0}, {c0 + n})")

            a_op = [
                pieces[0][0][:, g : g + 1].broadcast_to([P, CH]) for g in range(G)
            ]

            def b_slice(kq, g):
                return col_slice(AV + kq * BLK + CH + g * P, P)

            def x_chunk(kq, ni):
                if ni == 0:
                    return col_slice(AV + kq * BLK, CH)
                return col_slice(XB + kq * (W - CH) + (ni - 1) * CH, CH)

            h_t = [
                hpool.tile([P, W], bf16, tag=f"h{g}", name=f"h{g}")
                for g in range(G)
            ]

            def scan_and_store(g, ni, ps_g):
                n0 = ni * CH
                init = 0.0 if ni == 0 else h_t[g][:, n0 - 1 : n0]
                nc.vector.tensor_tensor_scan(
                    h_t[g][:, n0 : n0 + CH],
                    a_op[g],
                    ps_g[:],
                    init,
                    op0=mybir.AluOpType.mult,
                    op1=mybir.AluOpType.add,
                )
                lo = HALO if ni == 0 else 0
                eng = nc.scalar if g % 2 == 0 else nc.sync
                eng.dma_start(
                    out_e[g * P : (g + 1) * P, n0 + lo - HALO : n0 + CH - HALO],
                    h_t[g][:, n0 + lo : n0 + CH],
                )

            # chunk 0: kq-major across all 8 PSUM banks (DMA-arrival
            # matched).  Note: a staggered per-group (kq5,6,7) finishing
            # burst was tried to start the scans earlier — it measured
            # 2.6 us WORSE (the bursts stall on late b arrivals where the
            # dense pass order rides just behind them).
            ps0 = [
                pspool.tile([P, CH], f32, tag="ps", name=f"ps0_{g}")
                for g in range(G)
            ]
            for kq in range(KQ):
                for g in range(G):
                    nc.tensor.matmul(
                        ps0[g][:],
                        b_slice(kq, g),
                        x_chunk(kq, 0),
                        start=(kq == 0),
                        stop=(kq == KQ - 1),
                    )
            for g in range(G):
                scan_and_store(g, 0, ps0[g])

            # chunks 1-2: g-major, scans chase and free banks just in time
            for ni in (1, 2):
                for g in range(G):
                    ps_g = pspool.tile([P, CH], f32, tag="ps", name=f"ps{ni}_{g}")
                    for kq in range(KQ):
                        nc.tensor.matmul(
                            ps_g[:],
                            b_slice(kq, g),
                            x_chunk(kq, ni),
                            start=(kq == 0),
                            stop=(kq == KQ - 1),
                        )
                    scan_and_store(g, ni, ps_g)

            # warm-MM flush store, late, on scalar's ring (anti-DCE)
            nc.scalar.dma_start(flush_e[:], flush_sb[:])

    nc.compile()
    return nc


def _get_nc():
    if "nc" not in _state:
        _state["nc"] = _build_nc()
    return _state["nc"]


def _shard_inputs(x_seq, a_diag, b_mat):
    x = np.asarray(x_seq, dtype=np.float32)
    a = np.asarray(a_diag, dtype=np.float32)
    b = np.asarray(b_mat, dtype=np.float32)
    bq = b.astype(ml_dtypes.bfloat16)  # [H, H]
    x_pad = np.concatenate([np.zeros((HALO, H), np.float32), x], axis=0)
    xT = np.ascontiguousarray(x_pad.T).astype(ml_dtypes.bfloat16)  # [H, T+HALO]
    avT = np.ascontiguousarray(a.reshape(G, P).T).astype(ml_dtypes.bfloat16)

    in_maps = []
    for i in range(NC):
        slab = xT[:, i * T_LOC : i * T_LOC + W]  # [H, W]
        xcat = np.empty((P, NCOL), dtype=ml_dtypes.bfloat16)
        xcat[:, 0:AV] = avT
        for kq in range(KQ):
            r0, r1 = kq * P, (kq + 1) * P
            c = AV + kq * BLK
            xcat[:, c : c + CH] = slab[r0:r1, 0:CH]
            xcat[:, c + CH : c + BLK] = bq[r0:r1, :]
            xc = XB + kq * (W - CH)
            xcat[:, xc : xc + (W - CH)] = slab[r0:r1, CH:W]
        in_maps.append({"xcat": xcat})
    return in_maps


def kernel(x_seq, a_diag, b_mat):
    from concourse.bass_utils import run_bass_kernel_spmd

    nc = _get_nc()
    in_maps = _shard_inputs(x_seq, a_diag, b_mat)
    res = run_bass_kernel_spmd(nc, in_maps, list(range(NC)))
    _state["last_result"] = res
    out = np.concatenate(
        [
            np.asarray(res.results[i]["out"]).astype(np.float32).T
            for i in range(NC)
        ],
        axis=0,
    )
    return out
